# revision 68
# baseline (speedup 1.0000x reference)
"""AttentionPool Trainium2 kernel v2: fp8 DoubleRow matmuls, host-side
pre-transpose, bf16 staging.

Reference computation (per batch b of 32, S=2048, D=1024):
    xn = LayerNorm(x[b])                      # over D, eps 1e-5
    h = tanh(xn @ W1 + b1)
    scores = h @ W2 + b2                      # [S]
    w = softmax(scores)
    out[b] = sum_s w[s] * x[b, s, :]

Strategy: batch axis sharded over 8 cores (4 batches each). Host stages
x twice in bf16: [s, d] layout (LN stats + pooling values) and [d, s]
layout (pre-transposed, feeds matmul1) — no on-device transposes. Host
folds ln_gamma into W1 and ln_beta@W1+b1 into c2, and scales W1/W2 by 64
so fp8e4 (e4m3) quantization stays in the normal range; the inverse
scales ride the ACT activation `scale` operand.

Per core, per batch:
  - LN stats on DVE (bn_stats/bn_aggr + Newton rsqrt) in [s,d] layout;
    mu and rstd*16 bounce through DRAM and are broadcast-loaded as
    [128, S] tiles (per-free-column vectors for the transposed layout).
  - T-space normalize on DVE: xn8 = (xT - mu_b) * rs_b  -> fp8e4,
    written as [128, 2, S] d-pair tiles (DoubleRow operand layout).
  - matmul1: fp8 DoubleRow (K=256 per instruction), PSUM accumulate,
    tanh+c2 on ACT -> fp8 h pair tiles; scores via fp8 DoubleRow,
    exp on ACT (accum_out gives Z per chunk).
  - pooling via bf16 matmuls against the [s,d] x staging tiles kept in
    SBUF; divide by Z at the end.
Engine queues: GpSimd = x[s,d] loads; Sync = xT loads + stat stores +
output; Scalar(ACT) = broadcast loads + e-scatter bounces.
"""
import sys
import os

sys.path.insert(0, '/opt/trn_rl_repo')

import numpy as np

import concourse.bass as bass
import concourse.tile as tile
from concourse import bacc, mybir, library_config
from concourse.bass_utils import run_bass_kernel_spmd

P = 128
D = 1024
S = 2048
B = 32
NCORES = 8
BLOC = B // NCORES            # batches per core
ROWS = BLOC * S               # 8192 rows per core
DT = D // P                   # 8 d-tiles
ET = D // P                   # 8 e-tiles
DP = DT // 2                  # 4 d-pairs (DoubleRow)
EP = ET // 2                  # 4 e-pairs
SUBT = S // P                 # 16 subtiles per batch
NG = 4                        # subtiles per stats group
CHUNK = 512                   # matmul moving free dim
NCHUNK = S // CHUNK           # 4 chunks per batch

SW = 64.0                     # W1/W2 fp8 pre-scale (host)
SX = 16.0                     # xn fp8 pre-scale (device)
MM1_SCALE = 1.0 / (SW * SX)   # applied in tanh activation
SC_SCALE = 1.0 / SW           # applied in exp activation

f32 = mybir.dt.float32
bf16 = mybir.dt.bfloat16
fp8 = mybir.dt.float8e4
AF = mybir.ActivationFunctionType
ALU = mybir.AluOpType
DR = mybir.MatmulPerfMode.DoubleRow
DRSWI = mybir.MatmulPerfMode.DoubleRowSwInterleave


def build_nc():
    nc = bacc.Bacc("TRN2", target_bir_lowering=False, num_devices=NCORES)

    # both x stagings are host-relayouted partition-major so every DMA load
    # is a few large contiguous runs per partition (descriptor-rate limits
    # the rings well below their byte bandwidth otherwise)
    xbf = nc.dram_tensor("xbf", [P, BLOC * SUBT * D], bf16,
                         kind="ExternalInput")
    xt = nc.dram_tensor("xt", [P, BLOC * DP * 2 * S], bf16,
                        kind="ExternalInput")
    # W1 pre-interleaved on host for DoubleRowSwInterleave ldweights:
    # per partition d_p, free dim = [A_127 B_127 ... A_0 B_0] per (pair, e)
    w1q = nc.dram_tensor("w1q", [P, DP * ET * 2 * P], fp8,
                         kind="ExternalInput")
    # host-relayouted partition-major: [P, ET] so const loads are contiguous
    w2q = nc.dram_tensor("w2q", [P, ET], fp8, kind="ExternalInput")
    c2v = nc.dram_tensor("c2v", [P, ET], f32, kind="ExternalInput")
    b2s = nc.dram_tensor("b2s", [1, 1], f32, kind="ExternalInput")
    eye = nc.dram_tensor("eye", [P, P], bf16, kind="ExternalInput")
    out = nc.dram_tensor("out", [BLOC, D], f32, kind="ExternalOutput")

    with tile.TileContext(nc) as tc:
        with (
            tc.tile_pool(name="consts", bufs=1) as consts,
            tc.tile_pool(name="xb", bufs=2) as xbp,            # [128,16,1024] bf16
            tc.tile_pool(name="stats", bufs=8) as statp,
            tc.tile_pool(name="bcast", bufs=8) as bcp,         # [128,512] bf16
            tc.tile_pool(name="xtp", bufs=10) as xtpp,         # [128,2,1024] bf16
            tc.tile_pool(name="xn8", bufs=16) as xn8p,         # [128,2,1024] fp8
            tc.tile_pool(name="h8", bufs=8) as h8p,            # [128,2,512] fp8
            tc.tile_pool(name="ec", bufs=4) as ecp,            # [1,512] bf16
            tc.tile_pool(name="epk", bufs=8) as epkp,          # [128,4] bf16
            tc.tile_pool(name="z", bufs=4) as zp,              # tiny scalars
            tc.tile_pool(name="pack", bufs=3) as packp,        # [2,1024] bf16
            tc.tile_pool(name="ob", bufs=2) as obp,
            tc.tile_pool(name="psmm", bufs=4, space="PSUM") as psmm,
            tc.tile_pool(name="pssc", bufs=1, space="PSUM") as pssc,
            tc.tile_pool(name="pspool", bufs=2, space="PSUM") as pspool,
            tc.tile_pool(name="pst", bufs=1, space="PSUM") as pstp,
            tc.tile_pool(name="dram", bufs=8, space="DRAM") as dramp,
        ):
            # ---- constants ----
            w1_sb = consts.tile([P, DP, ET, 2 * P], fp8)   # interleaved pairs
            w1v = w1q.ap().rearrange("p (i e m) -> p i e m", i=DP, e=ET)
            for i in range(DP):     # split so the first matmul gates on 256KB
                nc.scalar.dma_start(w1_sb[:, i, :, :], w1v[:, i, :, :])
            # dual-fp8 ldweights needs a 16B-aligned outer free step: pad
            # each e-tile's single weight column out to 16 bytes
            w2_sb = consts.tile([P, ET, 16], fp8)
            nc.scalar.dma_start(w2_sb[:, :, 0:1], w2q.ap().unsqueeze(2))
            c2_sb = consts.tile([P, ET], f32)
            nc.scalar.dma_start(c2_sb, c2v.ap())
            b2_sb = consts.tile([1, 1], f32)
            nc.sync.dma_start(b2_sb, b2s.ap())
            eye_sb = consts.tile([P, P], bf16)
            nc.sync.dma_start(eye_sb, eye.ap())

            xbf4 = xbf.ap().rearrange("p (b t d) -> p b t d", b=BLOC, t=SUBT)
            xt5 = xt.ap().rearrange("p (b i u s) -> p b i u s",
                                    b=BLOC, i=DP, u=2)

            HB = S // 2           # half-batch token granularity
            HSUB = SUBT // 2      # 8 subtiles per half

            def phase1_loads(b):
                """Queue all of batch b's loads, spread across all three DMA
                rings (~2.7MB each per batch) — one ring at ~55GB/s cannot
                carry a 4MB stream per batch without pacing the pipeline.
                Priority: xb g0/g1 and xT-h0 first (they gate stats h0 and
                the first matmuls)."""
                xb = xbp.tile([P, SUBT, D], bf16, tag="xb")
                xtps = [[None] * DP, [None] * DP]

                def xb_load(q, g):
                    tg = g * NG
                    q.dma_start(xb[:, tg:tg + NG, :], xbf4[:, b, tg:tg + NG, :])

                def xt_load(q, h, i):
                    xtp = xtpp.tile([P, 2, HB], bf16, tag="xtp", name="xtp")
                    q.dma_start(xtp, xt5[:, b, i, :, h * HB:(h + 1) * HB])
                    xtps[h][i] = xtp

                xb_load(nc.gpsimd, 0)
                for i in range(DP):
                    xt_load(nc.sync, 0, i)        # xT h0: sync (2MB)
                xb_load(nc.gpsimd, 1)             # xb g0,g1,g2: gpsimd
                xb_load(nc.gpsimd, 2)
                xb_load(nc.sync, 3)               # xb g3: sync
                xt_load(nc.gpsimd, 1, 0)          # xT h1: gpsimd + scalar
                xt_load(nc.gpsimd, 1, 1)
                xt_load(nc.scalar, 1, 2)
                xt_load(nc.scalar, 1, 3)
                statd = dramp.tile([2, S], bf16, tag="statd", name="statd")
                return xb, xtps, statd

            def phase1_stats(b, xb, statd, h):
                """LN stats + quake+1-Newton rsqrt for half h; PE-transpose
                the packed mu|rstd*SX tile so the DRAM bounce store is
                contiguous, then broadcast-load as [128, HB] tiles."""
                t0 = h * HSUB
                mvh = statp.tile([P, HSUB, 2], f32, tag="mvh")
                for s in range(HSUB):
                    st = statp.tile([P, 2, 6], f32, tag="bnst")
                    nc.vector.bn_stats(st[:, 0, :], xb[:, t0 + s, 0:512])
                    nc.vector.bn_stats(st[:, 1, :], xb[:, t0 + s, 512:1024])
                    nc.vector.bn_aggr(mvh[:, s, :], st)
                var = statp.tile([P, HSUB], f32, tag="var")
                nc.vector.tensor_scalar(out=var, in0=mvh[:, :, 1],
                                        scalar1=1e-5, scalar2=0.5,
                                        op0=ALU.add, op1=ALU.mult)
                y = statp.tile([P, HSUB], f32, tag="y")
                yi = y.bitcast(mybir.dt.int32)
                vi = var.bitcast(mybir.dt.int32)
                nc.vector.tensor_scalar(out=yi, in0=vi, scalar1=0x800000,
                                        scalar2=None, op0=ALU.add)
                nc.vector.tensor_scalar(out=yi, in0=yi, scalar1=1,
                                        scalar2=None,
                                        op0=ALU.logical_shift_right)
                nc.vector.tensor_scalar(out=yi, in0=yi, scalar1=-1,
                                        scalar2=0x5f3759df,
                                        op0=ALU.mult, op1=ALU.add)
                tny = statp.tile([P, HSUB], f32, tag="tny")
                nc.vector.tensor_tensor(tny, y, y, ALU.mult)
                nc.vector.tensor_tensor(tny, tny, var, ALU.mult)
                nc.vector.tensor_scalar(out=tny, in0=tny, scalar1=-1.0,
                                        scalar2=1.5,
                                        op0=ALU.mult, op1=ALU.add)
                nc.vector.tensor_tensor(y, y, tny, ALU.mult)
                mr = statp.tile([P, 2 * HSUB], bf16, tag="mr")
                nc.vector.tensor_copy(mr[:, 0:HSUB], mvh[:, :, 0])
                nc.vector.tensor_scalar(out=mr[:, HSUB:2 * HSUB], in0=y,
                                        scalar1=SX, scalar2=None,
                                        op0=ALU.mult)
                mrt = pstp.tile([2 * HSUB, P], bf16, tag="mrt")
                nc.tensor.transpose(mrt, mr, eye_sb)
                mrs = statp.tile([2 * HSUB, P], bf16, tag="mrs")
                nc.scalar.activation(mrs, mrt, AF.Copy)
                hs = slice(h * HB, (h + 1) * HB)
                nc.gpsimd.dma_start(statd[0:2, hs], mrs)
                mu_bh = bcp.tile([P, HB], bf16, tag="mu_b", name="mu_b")
                nc.sync.dma_start(
                    mu_bh, statd[0:1, hs].to_broadcast((P, HB)))
                rs_bh = bcp.tile([P, HB], bf16, tag="rs_b", name="rs_b")
                nc.sync.dma_start(
                    rs_bh, statd[1:2, hs].to_broadcast((P, HB)))
                return mu_bh, rs_bh

            def phase2_half(b, xtps, bc, h):
                """T-space normalize half h to fp8 pair tiles."""
                mu_bh, rs_bh = bc
                xn8s = []
                for i in range(DP):
                    xn8 = xn8p.tile([P, 2, HB], fp8, tag="xn8", name="xn8")
                    for j in range(2):
                        nc.vector.tensor_tensor(xtps[h][i][:, j],
                                                xtps[h][i][:, j], mu_bh,
                                                ALU.subtract)
                        nc.vector.tensor_tensor(xn8[:, j],
                                                xtps[h][i][:, j], rs_bh,
                                                ALU.mult)
                    xn8s.append(xn8)
                return xn8s

            def phase3_half(b, xn8s_h, h, zc):
                """fp8 DoubleRow matmul1 + tanh + scores + exp for half h."""
                epks = []
                for cc in range(2):
                    c = h * 2 + cc
                    cs = slice(cc * CHUNK, (cc + 1) * CHUNK)
                    h8s = [h8p.tile([P, 2, CHUNK], fp8, tag="h8", name="h8")
                           for _ in range(EP)]
                    for e in range(ET):
                        ps = psmm.tile([P, CHUNK], f32, tag="psmm")
                        for i in range(DP):
                            nc.tensor.matmul(
                                ps, w1_sb[:, i, e, :],
                                xn8s_h[i][:, :, cs],
                                start=(i == 0), stop=(i == DP - 1),
                                perf_mode=DRSWI)
                        nc.scalar.activation(h8s[e // 2][:, e % 2, :], ps,
                                             AF.Tanh, bias=c2_sb[:, e:e + 1],
                                             scale=MM1_SCALE)
                    ps_sc = pssc.tile([1, CHUNK], f32, tag="pssc")
                    for k in range(EP):
                        nc.tensor.matmul(ps_sc,
                                         w2_sb[:, 2 * k:2 * k + 2, 0:1],
                                         h8s[k], start=(k == 0),
                                         stop=(k == EP - 1), perf_mode=DR)
                    ec = ecp.tile([1, CHUNK], bf16, tag="ec", name="ec")
                    nc.scalar.activation(ec, ps_sc, AF.Exp,
                                         bias=b2_sb[0:1, 0:1], scale=SC_SCALE,
                                         accum_out=zc[:, c:c + 1])
                    eb = dramp.tile([1, CHUNK], bf16, tag="eb", name="eb")
                    nc.scalar.dma_start(eb, ec)
                    epk = epkp.tile([P, NCHUNK], bf16, tag="epk", name="epk")
                    nc.scalar.dma_start(
                        epk, eb.rearrange("o (t p) -> (o p) t", p=P))
                    epks.append(epk)
                return epks

            def pool_half(xb, epks2, pp0, pp1, h):
                """Pooling matmuls for one half's chunks (one accumulation
                group spanning both halves: start at tt==0, stop at 15)."""
                for cc in range(2):
                    c = h * 2 + cc
                    for t in range(NG):
                        tt = c * NG + t
                        nc.tensor.matmul(pp0, epks2[cc][:, t:t + 1],
                                         xb[:, tt, 0:512],
                                         start=(tt == 0), stop=(tt == SUBT - 1))
                        nc.tensor.matmul(pp1, epks2[cc][:, t:t + 1],
                                         xb[:, tt, 512:1024],
                                         start=(tt == 0), stop=(tt == SUBT - 1))

            def phase4(b, xb, zc, epks, pps=None):
                """Pooling matmuls vs SBUF-kept x[s,d], divide by Z, store."""
                if pps is None:
                    pp0 = pspool.tile([1, CHUNK], f32, tag="pspool", name="pp0")
                    pp1 = pspool.tile([1, CHUNK], f32, tag="pspool", name="pp1")
                    pool_half(xb, epks[0:2], pp0, pp1, 0)
                    pool_half(xb, epks[2:4], pp0, pp1, 1)
                else:
                    pp0, pp1 = pps
                zt = zp.tile([1, 1], f32, tag="zt")
                nc.vector.tensor_reduce(zt, zc,
                                        axis=mybir.AxisListType.X, op=ALU.add)
                rz = zp.tile([1, 1], f32, tag="rz")
                nc.vector.reciprocal(rz, zt)
                ob = obp.tile([1, D], f32, tag="ob")
                nc.scalar.activation(ob[:, 0:512], pp0, AF.Copy,
                                     scale=rz[0:1, 0:1])
                nc.scalar.activation(ob[:, 512:1024], pp1, AF.Copy,
                                     scale=rz[0:1, 0:1])
                nc.sync.dma_start(out.ap()[b:b + 1, :], ob)

            prev = None
            for b in range(BLOC):
                last = False      # A/B: last-batch pooling split was neutral
                xb, xtps, statd = phase1_loads(b)
                bc0 = phase1_stats(b, xb, statd, 0)
                bc1 = phase1_stats(b, xb, statd, 1)
                if prev is not None:
                    phase4(*prev)
                zc = zp.tile([1, NCHUNK], f32, tag="zc", name="zc")
                xn8_h0 = phase2_half(b, xtps, bc0, 0)
                epks = phase3_half(b, xn8_h0, 0, zc)
                xn8_h1 = phase2_half(b, xtps, bc1, 1)
                if last:
                    # drain the last batch faster: pool h0 while h1 computes
                    pp0 = pspool.tile([1, CHUNK], f32, tag="pspool",
                                      name="pp0")
                    pp1 = pspool.tile([1, CHUNK], f32, tag="pspool",
                                      name="pp1")
                    pool_half(xb, epks, pp0, pp1, 0)
                epks += phase3_half(b, xn8_h1, 1, zc)
                if last:
                    pool_half(xb, epks[2:4], pp0, pp1, 1)
                    prev = (b, xb, zc, epks, (pp0, pp1))
                else:
                    prev = (b, xb, zc, epks)
            phase4(*prev)

    nc.compile()
    return nc


_NC_CACHE = {}


def _get_nc():
    if "nc" not in _NC_CACHE:
        _NC_CACHE["nc"] = build_nc()
    return _NC_CACHE["nc"]


def _prep_host(ln_gamma, ln_beta, W1, b1, W2, b2):
    import ml_dtypes
    f8 = ml_dtypes.float8_e4m3fn
    W1p = (np.asarray(ln_gamma, np.float32)[:, None]
           * np.asarray(W1, np.float32))
    w1q = np.clip(W1p * SW, -448, 448).astype(f8)
    # interleave for DoubleRowSwInterleave: per (pair i, e-tile) the 256
    # weight bytes per partition are [A_127 B_127 ... A_0 B_0] where
    # A/B are the even/odd d-tiles of the pair and columns are reversed
    b4 = w1q.reshape(DT, P, ET, P)            # [t, p, e, m]
    A, Bm = b4[0::2], b4[1::2]                # [DP, p, e, m]
    wv = np.empty((DP, P, ET, 2 * P), f8)
    wv[..., 0::2] = A[..., ::-1]
    wv[..., 1::2] = Bm[..., ::-1]
    w1s = np.ascontiguousarray(
        wv.transpose(1, 0, 2, 3).reshape(P, DP * ET * 2 * P))
    c2 = (np.asarray(ln_beta, np.float32) @ np.asarray(W1, np.float32)
          + np.asarray(b1, np.float32))
    c2pm = np.ascontiguousarray(c2.reshape(ET, P).T)          # [P, ET]
    w2q = np.clip(np.asarray(W2, np.float32)[:, 0] * SW,
                  -448, 448).astype(f8)
    w2pm = np.ascontiguousarray(w2q.reshape(ET, P).T)         # [P, ET]
    b2s = np.asarray(b2, np.float32).reshape(1, 1)
    return w1s, c2pm, w2pm, b2s


def run_cores(inputs, trace=False, **kw):
    import ml_dtypes
    x = np.asarray(inputs["x"], np.float32)
    w1q, c2, w2q, b2s = _prep_host(inputs["ln_gamma"], inputs["ln_beta"],
                                   inputs["W1"], inputs["b1"],
                                   inputs["W2"], inputs["b2"])
    xb16 = x.astype(ml_dtypes.bfloat16)          # [B, S, D]
    xt16 = xb16.transpose(0, 2, 1)               # [B, D, S] view
    nc = _get_nc()
    in_maps = []
    for c in range(NCORES):
        cb = slice(c * BLOC, (c + 1) * BLOC)
        # partition-major relayouts: [P, b, t, d] and [P, b, i, u, s]
        shard = np.ascontiguousarray(
            xb16[cb].reshape(BLOC, SUBT, P, D).transpose(2, 0, 1, 3)
        ).reshape(P, BLOC * SUBT * D)
        shardT = np.ascontiguousarray(
            xt16[cb].reshape(BLOC, DP, 2, P, S).transpose(3, 0, 1, 2, 4)
        ).reshape(P, BLOC * DP * 2 * S)
        in_maps.append(dict(xbf=shard, xt=shardT, w1q=w1q, w2q=w2q,
                            c2v=c2, b2s=b2s,
                            eye=np.eye(P, dtype=ml_dtypes.bfloat16)))
    res = run_bass_kernel_spmd(nc, in_maps, core_ids=list(range(NCORES)),
                               trace=trace, **kw)
    full = np.concatenate([res.results[c]["out"] for c in range(NCORES)], axis=0)
    return full, res


def kernel(**inputs) -> np.ndarray:
    out, _ = run_cores(inputs, trace=False)
    return out.astype(np.float32)


# revision 69
# speedup vs baseline: 1.0097x; 1.0097x over previous
"""AttentionPool Trainium2 kernel: fp8 dual-row matmuls, host-side
pre-transpose, bf16 staging.  ~306us HW exec (baseline was 534us).

Reference computation (per batch b of 32, S=2048, D=1024):
    xn = LayerNorm(x[b])                      # over D, eps 1e-5
    h = tanh(xn @ W1 + b1)
    scores = h @ W2 + b2                      # [S]
    w = softmax(scores)
    out[b] = sum_s w[s] * x[b, s, :]

Strategy: batch axis sharded over 8 cores (4 batches each). Host stages
x twice in bf16, both partition-major so every DMA is a few large
contiguous runs per partition (the DMA rings are descriptor-rate bound
otherwise): [s, d] layout (LN stats + pooling values) and [d, s] layout
(pre-transposed, feeds matmul1) — no on-device transposes. Host folds
ln_gamma into W1 and ln_beta@W1+b1 into c2, scales W1/W2 by 64 so fp8e4
(e4m3) quantization stays in the normal range (inverse scales ride the
ACT activation `scale` operand), and pre-interleaves W1 columns for
DoubleRowSwInterleave weight loads.

Per core, per half-batch (1024 tokens — halves pipeline the broadcast
round-trip under the other half's stats):
  - LN stats on DVE (bn_stats/bn_aggr + quake+1-Newton rsqrt) in [s,d]
    layout; mu | rstd*16 packed [128,16], PE-transposed so the DRAM
    bounce store is contiguous, then broadcast-loaded as [128, 1024]
    tiles (per-free-column vectors for the transposed layout).
  - T-space normalize on DVE: xn8 = (xT - mu_b) * rs_b  -> fp8e4
    [128, 2, 1024] d-pair tiles (dual-row operand layout, K=256/inst).
  - matmul1 at 2x PE rate (fp8 dual-row, 4 insts per 128x512 psum),
    tanh+c2 on ACT -> fp8 h pair tiles; scores via fp8 DoubleRow; exp
    on ACT (accum_out accumulates Z per chunk).
  - pooling via bf16 matmuls against the [s,d] x staging tiles kept in
    SBUF; divide by Z at the end.
PE weight loads (LDWEIGHTS) fully overlap matmuls (shadow registers).
Loads are spread over all three DMA rings (gpsimd/sync/scalar) at
~2.7-3MB per batch each; latency-critical small transfers (stat bounce,
broadcasts) are placed so they never sit behind bulk loads or
semaphore-stalled bounce DMAs at a ring head.
"""
import sys
import os

sys.path.insert(0, '/opt/trn_rl_repo')

import numpy as np

import concourse.bass as bass
import concourse.tile as tile
from concourse import bacc, mybir, library_config
from concourse.bass_utils import run_bass_kernel_spmd

P = 128
D = 1024
S = 2048
B = 32
NCORES = 8
BLOC = B // NCORES            # batches per core
ROWS = BLOC * S               # 8192 rows per core
DT = D // P                   # 8 d-tiles
ET = D // P                   # 8 e-tiles
DP = DT // 2                  # 4 d-pairs (DoubleRow)
EP = ET // 2                  # 4 e-pairs
SUBT = S // P                 # 16 subtiles per batch
NG = 4                        # subtiles per stats group
CHUNK = 512                   # matmul moving free dim
NCHUNK = S // CHUNK           # 4 chunks per batch

SW = 64.0                     # W1/W2 fp8 pre-scale (host)
SX = 16.0                     # xn fp8 pre-scale (device)
MM1_SCALE = 1.0 / (SW * SX)   # applied in tanh activation
SC_SCALE = 1.0 / SW           # applied in exp activation

f32 = mybir.dt.float32
bf16 = mybir.dt.bfloat16
fp8 = mybir.dt.float8e4
AF = mybir.ActivationFunctionType
ALU = mybir.AluOpType
DR = mybir.MatmulPerfMode.DoubleRow
DRSWI = mybir.MatmulPerfMode.DoubleRowSwInterleave


def build_nc():
    nc = bacc.Bacc("TRN2", target_bir_lowering=False, num_devices=NCORES)

    # both x stagings are host-relayouted partition-major so every DMA load
    # is a few large contiguous runs per partition (descriptor-rate limits
    # the rings well below their byte bandwidth otherwise)
    xbf = nc.dram_tensor("xbf", [P, BLOC * SUBT * D], bf16,
                         kind="ExternalInput")
    xt = nc.dram_tensor("xt", [P, BLOC * DP * 2 * S], bf16,
                        kind="ExternalInput")
    # W1 pre-interleaved on host for DoubleRowSwInterleave ldweights:
    # per partition d_p, free dim = [A_127 B_127 ... A_0 B_0] per (pair, e)
    w1q = nc.dram_tensor("w1q", [P, DP * ET * 2 * P], fp8,
                         kind="ExternalInput")
    # host-relayouted partition-major: [P, ET] so const loads are contiguous
    w2q = nc.dram_tensor("w2q", [P, ET], fp8, kind="ExternalInput")
    c2v = nc.dram_tensor("c2v", [P, ET], f32, kind="ExternalInput")
    b2s = nc.dram_tensor("b2s", [1, 1], f32, kind="ExternalInput")
    eye = nc.dram_tensor("eye", [P, P], bf16, kind="ExternalInput")
    out = nc.dram_tensor("out", [BLOC, D], f32, kind="ExternalOutput")

    with tile.TileContext(nc) as tc:
        with (
            tc.tile_pool(name="consts", bufs=1) as consts,
            tc.tile_pool(name="xb", bufs=2) as xbp,            # [128,16,1024] bf16
            tc.tile_pool(name="stats", bufs=8) as statp,
            tc.tile_pool(name="bcast", bufs=8) as bcp,         # [128,512] bf16
            tc.tile_pool(name="xtp", bufs=10) as xtpp,         # [128,2,1024] bf16
            tc.tile_pool(name="xn8", bufs=16) as xn8p,         # [128,2,1024] fp8
            tc.tile_pool(name="h8", bufs=8) as h8p,            # [128,2,512] fp8
            tc.tile_pool(name="ec", bufs=4) as ecp,            # [1,512] bf16
            tc.tile_pool(name="epk", bufs=8) as epkp,          # [128,4] bf16
            tc.tile_pool(name="z", bufs=4) as zp,              # tiny scalars
            tc.tile_pool(name="pack", bufs=3) as packp,        # [2,1024] bf16
            tc.tile_pool(name="ob", bufs=2) as obp,
            tc.tile_pool(name="psmm", bufs=4, space="PSUM") as psmm,
            tc.tile_pool(name="pssc", bufs=1, space="PSUM") as pssc,
            tc.tile_pool(name="pspool", bufs=2, space="PSUM") as pspool,
            tc.tile_pool(name="pst", bufs=1, space="PSUM") as pstp,
            tc.tile_pool(name="dram", bufs=8, space="DRAM") as dramp,
        ):
            # ---- constants ----
            w1_sb = consts.tile([P, DP, ET, 2 * P], fp8)   # interleaved pairs
            w1v = w1q.ap().rearrange("p (i e m) -> p i e m", i=DP, e=ET)
            for i in range(DP):     # split so the first matmul gates on 256KB
                nc.scalar.dma_start(w1_sb[:, i, :, :], w1v[:, i, :, :])
            # dual-fp8 ldweights needs a 16B-aligned outer free step: pad
            # each e-tile's single weight column out to 16 bytes
            w2_sb = consts.tile([P, ET, 16], fp8)
            nc.scalar.dma_start(w2_sb[:, :, 0:1], w2q.ap().unsqueeze(2))
            c2_sb = consts.tile([P, ET], f32)
            nc.scalar.dma_start(c2_sb, c2v.ap())
            b2_sb = consts.tile([1, 1], f32)
            nc.sync.dma_start(b2_sb, b2s.ap())
            eye_sb = consts.tile([P, P], bf16)
            nc.sync.dma_start(eye_sb, eye.ap())

            xbf4 = xbf.ap().rearrange("p (b t d) -> p b t d", b=BLOC, t=SUBT)
            xt5 = xt.ap().rearrange("p (b i u s) -> p b i u s",
                                    b=BLOC, i=DP, u=2)

            HB = S // 2           # half-batch token granularity
            HSUB = SUBT // 2      # 8 subtiles per half

            def phase1_loads(b):
                """Queue all of batch b's loads, spread across all three DMA
                rings (~2.7MB each per batch) — one ring at ~55GB/s cannot
                carry a 4MB stream per batch without pacing the pipeline.
                Priority: xb g0/g1 and xT-h0 first (they gate stats h0 and
                the first matmuls)."""
                xb = xbp.tile([P, SUBT, D], bf16, tag="xb")
                xtps = [[None] * DP, [None] * DP]

                def xb_load(q, g):
                    tg = g * NG
                    q.dma_start(xb[:, tg:tg + NG, :], xbf4[:, b, tg:tg + NG, :])

                def xt_load(q, h, i):
                    xtp = xtpp.tile([P, 2, HB], bf16, tag="xtp", name="xtp")
                    q.dma_start(xtp, xt5[:, b, i, :, h * HB:(h + 1) * HB])
                    xtps[h][i] = xtp

                xb_load(nc.gpsimd, 0)
                for i in range(DP):
                    xt_load(nc.sync, 0, i)        # xT h0: sync (2MB)
                xb_load(nc.gpsimd, 1)             # xb g0,g1,g2: gpsimd
                xb_load(nc.gpsimd, 2)
                xb_load(nc.sync, 3)               # xb g3: sync
                xt_load(nc.gpsimd, 1, 0)          # xT h1: gpsimd + scalar
                xt_load(nc.gpsimd, 1, 1)
                xt_load(nc.scalar, 1, 2)
                xt_load(nc.scalar, 1, 3)
                statd = dramp.tile([2, S], bf16, tag="statd", name="statd")
                return xb, xtps, statd

            def phase1_stats(b, xb, statd, h):
                """LN stats + quake+1-Newton rsqrt for half h; PE-transpose
                the packed mu|rstd*SX tile so the DRAM bounce store is
                contiguous, then broadcast-load as [128, HB] tiles."""
                t0 = h * HSUB
                mvh = statp.tile([P, HSUB, 2], f32, tag="mvh")
                for s in range(HSUB):
                    st = statp.tile([P, 2, 6], f32, tag="bnst")
                    nc.vector.bn_stats(st[:, 0, :], xb[:, t0 + s, 0:512])
                    nc.vector.bn_stats(st[:, 1, :], xb[:, t0 + s, 512:1024])
                    nc.vector.bn_aggr(mvh[:, s, :], st)
                var = statp.tile([P, HSUB], f32, tag="var")
                nc.vector.tensor_scalar(out=var, in0=mvh[:, :, 1],
                                        scalar1=1e-5, scalar2=0.5,
                                        op0=ALU.add, op1=ALU.mult)
                y = statp.tile([P, HSUB], f32, tag="y")
                yi = y.bitcast(mybir.dt.int32)
                vi = var.bitcast(mybir.dt.int32)
                nc.vector.tensor_scalar(out=yi, in0=vi, scalar1=0x800000,
                                        scalar2=None, op0=ALU.add)
                nc.vector.tensor_scalar(out=yi, in0=yi, scalar1=1,
                                        scalar2=None,
                                        op0=ALU.logical_shift_right)
                nc.vector.tensor_scalar(out=yi, in0=yi, scalar1=-1,
                                        scalar2=0x5f3759df,
                                        op0=ALU.mult, op1=ALU.add)
                tny = statp.tile([P, HSUB], f32, tag="tny")
                nc.vector.tensor_tensor(tny, y, y, ALU.mult)
                nc.vector.tensor_tensor(tny, tny, var, ALU.mult)
                nc.vector.tensor_scalar(out=tny, in0=tny, scalar1=-1.0,
                                        scalar2=1.5,
                                        op0=ALU.mult, op1=ALU.add)
                nc.vector.tensor_tensor(y, y, tny, ALU.mult)
                mr = statp.tile([P, 2 * HSUB], bf16, tag="mr")
                nc.vector.tensor_copy(mr[:, 0:HSUB], mvh[:, :, 0])
                nc.vector.tensor_scalar(out=mr[:, HSUB:2 * HSUB], in0=y,
                                        scalar1=SX, scalar2=None,
                                        op0=ALU.mult)
                mrt = pstp.tile([2 * HSUB, P], bf16, tag="mrt")
                nc.tensor.transpose(mrt, mr, eye_sb)
                mrs = statp.tile([2 * HSUB, P], bf16, tag="mrs")
                nc.scalar.activation(mrs, mrt, AF.Copy)
                hs = slice(h * HB, (h + 1) * HB)
                nc.gpsimd.dma_start(statd[0:2, hs], mrs)
                mu_bh = bcp.tile([P, HB], bf16, tag="mu_b", name="mu_b")
                nc.sync.dma_start(
                    mu_bh, statd[0:1, hs].to_broadcast((P, HB)))
                rs_bh = bcp.tile([P, HB], bf16, tag="rs_b", name="rs_b")
                nc.sync.dma_start(
                    rs_bh, statd[1:2, hs].to_broadcast((P, HB)))
                return mu_bh, rs_bh

            def phase2_half(b, xtps, bc, h):
                """T-space normalize half h to fp8 pair tiles."""
                mu_bh, rs_bh = bc
                xn8s = []
                for i in range(DP):
                    xn8 = xn8p.tile([P, 2, HB], fp8, tag="xn8", name="xn8")
                    for j in range(2):
                        nc.vector.tensor_tensor(xtps[h][i][:, j],
                                                xtps[h][i][:, j], mu_bh,
                                                ALU.subtract)
                        nc.vector.tensor_tensor(xn8[:, j],
                                                xtps[h][i][:, j], rs_bh,
                                                ALU.mult)
                    xn8s.append(xn8)
                return xn8s

            def phase3_half(b, xn8s_h, h, zc):
                """fp8 DoubleRow matmul1 + tanh + scores + exp for half h."""
                epks = []
                for cc in range(2):
                    c = h * 2 + cc
                    cs = slice(cc * CHUNK, (cc + 1) * CHUNK)
                    h8s = [h8p.tile([P, 2, CHUNK], fp8, tag="h8", name="h8")
                           for _ in range(EP)]
                    for e in range(ET):
                        ps = psmm.tile([P, CHUNK], f32, tag="psmm")
                        for i in range(DP):
                            nc.tensor.matmul(
                                ps, w1_sb[:, i, e, :],
                                xn8s_h[i][:, :, cs],
                                start=(i == 0), stop=(i == DP - 1),
                                perf_mode=DRSWI)
                        nc.scalar.activation(h8s[e // 2][:, e % 2, :], ps,
                                             AF.Tanh, bias=c2_sb[:, e:e + 1],
                                             scale=MM1_SCALE)
                    ps_sc = pssc.tile([1, CHUNK], f32, tag="pssc")
                    for k in range(EP):
                        nc.tensor.matmul(ps_sc,
                                         w2_sb[:, 2 * k:2 * k + 2, 0:1],
                                         h8s[k], start=(k == 0),
                                         stop=(k == EP - 1), perf_mode=DR)
                    ec = ecp.tile([1, CHUNK], bf16, tag="ec", name="ec")
                    nc.scalar.activation(ec, ps_sc, AF.Exp,
                                         bias=b2_sb[0:1, 0:1], scale=SC_SCALE,
                                         accum_out=zc[:, c:c + 1])
                    eb = dramp.tile([1, CHUNK], bf16, tag="eb", name="eb")
                    nc.scalar.dma_start(eb, ec)
                    epk = epkp.tile([P, NCHUNK], bf16, tag="epk", name="epk")
                    nc.scalar.dma_start(
                        epk, eb.rearrange("o (t p) -> (o p) t", p=P))
                    epks.append(epk)
                return epks

            def pool_half(xb, epks2, pp0, pp1, h):
                """Pooling matmuls for one half's chunks (one accumulation
                group spanning both halves: start at tt==0, stop at 15)."""
                for cc in range(2):
                    c = h * 2 + cc
                    for t in range(NG):
                        tt = c * NG + t
                        nc.tensor.matmul(pp0, epks2[cc][:, t:t + 1],
                                         xb[:, tt, 0:512],
                                         start=(tt == 0), stop=(tt == SUBT - 1))
                        nc.tensor.matmul(pp1, epks2[cc][:, t:t + 1],
                                         xb[:, tt, 512:1024],
                                         start=(tt == 0), stop=(tt == SUBT - 1))

            def phase4(b, xb, zc, epks, pps=None):
                """Pooling matmuls vs SBUF-kept x[s,d], divide by Z, store."""
                if pps is None:
                    pp0 = pspool.tile([1, CHUNK], f32, tag="pspool", name="pp0")
                    pp1 = pspool.tile([1, CHUNK], f32, tag="pspool", name="pp1")
                    pool_half(xb, epks[0:2], pp0, pp1, 0)
                    pool_half(xb, epks[2:4], pp0, pp1, 1)
                else:
                    pp0, pp1 = pps
                zt = zp.tile([1, 1], f32, tag="zt")
                nc.vector.tensor_reduce(zt, zc,
                                        axis=mybir.AxisListType.X, op=ALU.add)
                rz = zp.tile([1, 1], f32, tag="rz")
                nc.vector.reciprocal(rz, zt)
                ob = obp.tile([1, D], f32, tag="ob")
                nc.scalar.activation(ob[:, 0:512], pp0, AF.Copy,
                                     scale=rz[0:1, 0:1])
                nc.scalar.activation(ob[:, 512:1024], pp1, AF.Copy,
                                     scale=rz[0:1, 0:1])
                nc.sync.dma_start(out.ap()[b:b + 1, :], ob)

            prev = None
            for b in range(BLOC):
                last = False      # A/B: last-batch pooling split was neutral
                xb, xtps, statd = phase1_loads(b)
                bc0 = phase1_stats(b, xb, statd, 0)
                bc1 = phase1_stats(b, xb, statd, 1)
                if prev is not None:
                    phase4(*prev)
                zc = zp.tile([1, NCHUNK], f32, tag="zc", name="zc")
                xn8_h0 = phase2_half(b, xtps, bc0, 0)
                epks = phase3_half(b, xn8_h0, 0, zc)
                xn8_h1 = phase2_half(b, xtps, bc1, 1)
                if last:
                    # drain the last batch faster: pool h0 while h1 computes
                    pp0 = pspool.tile([1, CHUNK], f32, tag="pspool",
                                      name="pp0")
                    pp1 = pspool.tile([1, CHUNK], f32, tag="pspool",
                                      name="pp1")
                    pool_half(xb, epks, pp0, pp1, 0)
                epks += phase3_half(b, xn8_h1, 1, zc)
                if last:
                    pool_half(xb, epks[2:4], pp0, pp1, 1)
                    prev = (b, xb, zc, epks, (pp0, pp1))
                else:
                    prev = (b, xb, zc, epks)
            phase4(*prev)

    nc.compile()
    return nc


_NC_CACHE = {}


def _get_nc():
    if "nc" not in _NC_CACHE:
        _NC_CACHE["nc"] = build_nc()
    return _NC_CACHE["nc"]


def _prep_host(ln_gamma, ln_beta, W1, b1, W2, b2):
    import ml_dtypes
    f8 = ml_dtypes.float8_e4m3fn
    W1p = (np.asarray(ln_gamma, np.float32)[:, None]
           * np.asarray(W1, np.float32))
    w1q = np.clip(W1p * SW, -448, 448).astype(f8)
    # interleave for DoubleRowSwInterleave: per (pair i, e-tile) the 256
    # weight bytes per partition are [A_127 B_127 ... A_0 B_0] where
    # A/B are the even/odd d-tiles of the pair and columns are reversed
    b4 = w1q.reshape(DT, P, ET, P)            # [t, p, e, m]
    A, Bm = b4[0::2], b4[1::2]                # [DP, p, e, m]
    wv = np.empty((DP, P, ET, 2 * P), f8)
    wv[..., 0::2] = A[..., ::-1]
    wv[..., 1::2] = Bm[..., ::-1]
    w1s = np.ascontiguousarray(
        wv.transpose(1, 0, 2, 3).reshape(P, DP * ET * 2 * P))
    c2 = (np.asarray(ln_beta, np.float32) @ np.asarray(W1, np.float32)
          + np.asarray(b1, np.float32))
    c2pm = np.ascontiguousarray(c2.reshape(ET, P).T)          # [P, ET]
    w2q = np.clip(np.asarray(W2, np.float32)[:, 0] * SW,
                  -448, 448).astype(f8)
    w2pm = np.ascontiguousarray(w2q.reshape(ET, P).T)         # [P, ET]
    b2s = np.asarray(b2, np.float32).reshape(1, 1)
    return w1s, c2pm, w2pm, b2s


def run_cores(inputs, trace=False, **kw):
    import ml_dtypes
    x = np.asarray(inputs["x"], np.float32)
    w1q, c2, w2q, b2s = _prep_host(inputs["ln_gamma"], inputs["ln_beta"],
                                   inputs["W1"], inputs["b1"],
                                   inputs["W2"], inputs["b2"])
    xb16 = x.astype(ml_dtypes.bfloat16)          # [B, S, D]
    xt16 = xb16.transpose(0, 2, 1)               # [B, D, S] view
    nc = _get_nc()
    in_maps = []
    for c in range(NCORES):
        cb = slice(c * BLOC, (c + 1) * BLOC)
        # partition-major relayouts: [P, b, t, d] and [P, b, i, u, s]
        shard = np.ascontiguousarray(
            xb16[cb].reshape(BLOC, SUBT, P, D).transpose(2, 0, 1, 3)
        ).reshape(P, BLOC * SUBT * D)
        shardT = np.ascontiguousarray(
            xt16[cb].reshape(BLOC, DP, 2, P, S).transpose(3, 0, 1, 2, 4)
        ).reshape(P, BLOC * DP * 2 * S)
        in_maps.append(dict(xbf=shard, xt=shardT, w1q=w1q, w2q=w2q,
                            c2v=c2, b2s=b2s,
                            eye=np.eye(P, dtype=ml_dtypes.bfloat16)))
    res = run_bass_kernel_spmd(nc, in_maps, core_ids=list(range(NCORES)),
                               trace=trace, **kw)
    full = np.concatenate([res.results[c]["out"] for c in range(NCORES)], axis=0)
    return full, res


def kernel(**inputs) -> np.ndarray:
    out, _ = run_cores(inputs, trace=False)
    return out.astype(np.float32)


# revision 71
# speedup vs baseline: 1.0250x; 1.0151x over previous
"""AttentionPool Trainium2 kernel: fp8 dual-row matmuls, host-side
pre-transpose, bf16 staging.  ~306us HW exec (baseline was 534us).

Reference computation (per batch b of 32, S=2048, D=1024):
    xn = LayerNorm(x[b])                      # over D, eps 1e-5
    h = tanh(xn @ W1 + b1)
    scores = h @ W2 + b2                      # [S]
    w = softmax(scores)
    out[b] = sum_s w[s] * x[b, s, :]

Strategy: batch axis sharded over 8 cores (4 batches each). Host stages
x twice in bf16, both partition-major so every DMA is a few large
contiguous runs per partition (the DMA rings are descriptor-rate bound
otherwise): [s, d] layout (LN stats + pooling values) and [d, s] layout
(pre-transposed, feeds matmul1) — no on-device transposes. Host folds
ln_gamma into W1 and ln_beta@W1+b1 into c2, scales W1/W2 by 64 so fp8e4
(e4m3) quantization stays in the normal range (inverse scales ride the
ACT activation `scale` operand), and pre-interleaves W1 columns for
DoubleRowSwInterleave weight loads.

Per core, per half-batch (1024 tokens — halves pipeline the broadcast
round-trip under the other half's stats):
  - LN stats on DVE (bn_stats/bn_aggr + quake+1-Newton rsqrt) in [s,d]
    layout; mu | rstd*16 packed [128,16], PE-transposed so the DRAM
    bounce store is contiguous, then broadcast-loaded as [128, 1024]
    tiles (per-free-column vectors for the transposed layout).
  - T-space normalize on DVE: xn8 = (xT - mu_b) * rs_b  -> fp8e4
    [128, 2, 1024] d-pair tiles (dual-row operand layout, K=256/inst).
  - matmul1 at 2x PE rate (fp8 dual-row, 4 insts per 128x512 psum),
    tanh+c2 on ACT -> fp8 h pair tiles; scores via fp8 DoubleRow; exp
    on ACT (accum_out accumulates Z per chunk).
  - pooling via bf16 matmuls against the [s,d] x staging tiles kept in
    SBUF; divide by Z at the end.
PE weight loads (LDWEIGHTS) fully overlap matmuls (shadow registers).
Loads are spread over all three DMA rings (gpsimd/sync/scalar) at
~2.7-3MB per batch each; latency-critical small transfers (stat bounce,
broadcasts) are placed so they never sit behind bulk loads or
semaphore-stalled bounce DMAs at a ring head.
"""
import sys
import os

sys.path.insert(0, '/opt/trn_rl_repo')

import numpy as np

import concourse.bass as bass
import concourse.tile as tile
from concourse import bacc, mybir, library_config
from concourse.bass_utils import run_bass_kernel_spmd

P = 128
D = 1024
S = 2048
B = 32
NCORES = 8
BLOC = B // NCORES            # batches per core
ROWS = BLOC * S               # 8192 rows per core
DT = D // P                   # 8 d-tiles
ET = D // P                   # 8 e-tiles
DP = DT // 2                  # 4 d-pairs (DoubleRow)
EP = ET // 2                  # 4 e-pairs
SUBT = S // P                 # 16 subtiles per batch
NG = 4                        # subtiles per stats group
CHUNK = 512                   # matmul moving free dim
NCHUNK = S // CHUNK           # 4 chunks per batch

SW = 64.0                     # W1/W2 fp8 pre-scale (host)
SX = 16.0                     # xn fp8 pre-scale (device)
MM1_SCALE = 1.0 / (SW * SX)   # applied in tanh activation
SC_SCALE = 1.0 / SW           # applied in exp activation

f32 = mybir.dt.float32
bf16 = mybir.dt.bfloat16
fp8 = mybir.dt.float8e4
AF = mybir.ActivationFunctionType
ALU = mybir.AluOpType
DR = mybir.MatmulPerfMode.DoubleRow
DRSWI = mybir.MatmulPerfMode.DoubleRowSwInterleave


def build_nc():
    nc = bacc.Bacc("TRN2", target_bir_lowering=False, num_devices=NCORES)

    # both x stagings are host-relayouted partition-major so every DMA load
    # is a few large contiguous runs per partition (descriptor-rate limits
    # the rings well below their byte bandwidth otherwise)
    xbf = nc.dram_tensor("xbf", [P, BLOC * SUBT * D], bf16,
                         kind="ExternalInput")
    xt = nc.dram_tensor("xt", [P, BLOC * DP * 2 * S], bf16,
                        kind="ExternalInput")
    # W1 pre-interleaved on host for DoubleRowSwInterleave ldweights:
    # per partition d_p, free dim = [A_127 B_127 ... A_0 B_0] per (pair, e)
    w1q = nc.dram_tensor("w1q", [P, DP * ET * 2 * P], fp8,
                         kind="ExternalInput")
    # host-relayouted partition-major: [P, ET] so const loads are contiguous
    w2q = nc.dram_tensor("w2q", [P, ET], fp8, kind="ExternalInput")
    c2v = nc.dram_tensor("c2v", [P, ET], f32, kind="ExternalInput")
    b2s = nc.dram_tensor("b2s", [1, 1], f32, kind="ExternalInput")
    eye = nc.dram_tensor("eye", [P, P], bf16, kind="ExternalInput")
    out = nc.dram_tensor("out", [BLOC, D], f32, kind="ExternalOutput")

    with tile.TileContext(nc) as tc:
        with (
            tc.tile_pool(name="consts", bufs=1) as consts,
            tc.tile_pool(name="xb", bufs=2) as xbp,            # [128,16,1024] bf16
            tc.tile_pool(name="stats", bufs=8) as statp,
            tc.tile_pool(name="bcast", bufs=8) as bcp,         # [128,512] bf16
            tc.tile_pool(name="xtp", bufs=10) as xtpp,         # [128,2,1024] bf16
            tc.tile_pool(name="xn8", bufs=16) as xn8p,         # [128,2,1024] fp8
            tc.tile_pool(name="h8", bufs=8) as h8p,            # [128,2,512] fp8
            tc.tile_pool(name="ec", bufs=4) as ecp,            # [1,512] bf16
            tc.tile_pool(name="epk", bufs=8) as epkp,          # [128,4] bf16
            tc.tile_pool(name="z", bufs=4) as zp,              # tiny scalars
            tc.tile_pool(name="pack", bufs=3) as packp,        # [2,1024] bf16
            tc.tile_pool(name="ob", bufs=2) as obp,
            tc.tile_pool(name="psmm", bufs=4, space="PSUM") as psmm,
            tc.tile_pool(name="pssc", bufs=1, space="PSUM") as pssc,
            tc.tile_pool(name="pspool", bufs=2, space="PSUM") as pspool,
            tc.tile_pool(name="pst", bufs=1, space="PSUM") as pstp,
            tc.tile_pool(name="dram", bufs=8, space="DRAM") as dramp,
        ):
            # ---- constants ----
            w1_sb = consts.tile([P, DP, ET, 2 * P], fp8)   # interleaved pairs
            w1v = w1q.ap().rearrange("p (i e m) -> p i e m", i=DP, e=ET)
            for i in range(DP):     # split so the first matmul gates on 256KB
                nc.scalar.dma_start(w1_sb[:, i, :, :], w1v[:, i, :, :])
            # dual-fp8 ldweights needs a 16B-aligned outer free step: pad
            # each e-tile's single weight column out to 16 bytes
            w2_sb = consts.tile([P, ET, 16], fp8)
            nc.scalar.dma_start(w2_sb[:, :, 0:1], w2q.ap().unsqueeze(2))
            c2_sb = consts.tile([P, ET], f32)
            nc.scalar.dma_start(c2_sb, c2v.ap())
            b2_sb = consts.tile([1, 1], f32)
            nc.sync.dma_start(b2_sb, b2s.ap())
            eye_sb = consts.tile([P, P], bf16)
            nc.sync.dma_start(eye_sb, eye.ap())

            xbf4 = xbf.ap().rearrange("p (b t d) -> p b t d", b=BLOC, t=SUBT)
            xt5 = xt.ap().rearrange("p (b i u s) -> p b i u s",
                                    b=BLOC, i=DP, u=2)

            HB = S // 2           # half-batch token granularity
            HSUB = SUBT // 2      # 8 subtiles per half

            def phase1_loads(b):
                """Queue all of batch b's loads, spread across all three DMA
                rings (~2.7MB each per batch) — one ring at ~55GB/s cannot
                carry a 4MB stream per batch without pacing the pipeline.
                Priority: xb g0/g1 and xT-h0 first (they gate stats h0 and
                the first matmuls)."""
                xb = xbp.tile([P, SUBT, D], bf16, tag="xb")
                xtps = [[None] * DP, [None] * DP]

                def xb_load(q, g):
                    tg = g * NG
                    q.dma_start(xb[:, tg:tg + NG, :], xbf4[:, b, tg:tg + NG, :])

                def xt_load(q, h, i):
                    xtp = xtpp.tile([P, 2, HB], bf16, tag="xtp", name="xtp")
                    q.dma_start(xtp, xt5[:, b, i, :, h * HB:(h + 1) * HB])
                    xtps[h][i] = xtp

                xb_load(nc.gpsimd, 0)
                for i in range(DP):
                    xt_load(nc.sync, 0, i)        # xT h0: sync (2MB)
                xb_load(nc.gpsimd, 1)             # xb g0,g1,g2: gpsimd
                xb_load(nc.gpsimd, 2)
                xb_load(nc.sync, 3)               # xb g3: sync
                xt_load(nc.scalar, 1, 2)          # xT h1 p2/p3: scalar
                xt_load(nc.scalar, 1, 3)
                statd = dramp.tile([2, S], bf16, tag="statd", name="statd")

                def late_loads():
                    # emitted after the h0 stat store so that store (fires
                    # at Newton-h0) is not FIFO-queued behind 1MB of xT
                    xt_load(nc.gpsimd, 1, 0)
                    xt_load(nc.gpsimd, 1, 1)
                return xb, xtps, statd, late_loads

            def phase1_stats(b, xb, statd, h):
                """LN stats + quake+1-Newton rsqrt for half h; PE-transpose
                the packed mu|rstd*SX tile so the DRAM bounce store is
                contiguous, then broadcast-load as [128, HB] tiles."""
                t0 = h * HSUB
                mvh = statp.tile([P, HSUB, 2], f32, tag="mvh")
                for s in range(HSUB):
                    st = statp.tile([P, 2, 6], f32, tag="bnst")
                    nc.vector.bn_stats(st[:, 0, :], xb[:, t0 + s, 0:512])
                    nc.vector.bn_stats(st[:, 1, :], xb[:, t0 + s, 512:1024])
                    nc.vector.bn_aggr(mvh[:, s, :], st)
                var = statp.tile([P, HSUB], f32, tag="var")
                nc.vector.tensor_scalar(out=var, in0=mvh[:, :, 1],
                                        scalar1=1e-5, scalar2=0.5,
                                        op0=ALU.add, op1=ALU.mult)
                y = statp.tile([P, HSUB], f32, tag="y")
                yi = y.bitcast(mybir.dt.int32)
                vi = var.bitcast(mybir.dt.int32)
                nc.vector.tensor_scalar(out=yi, in0=vi, scalar1=0x800000,
                                        scalar2=None, op0=ALU.add)
                nc.vector.tensor_scalar(out=yi, in0=yi, scalar1=1,
                                        scalar2=None,
                                        op0=ALU.logical_shift_right)
                nc.vector.tensor_scalar(out=yi, in0=yi, scalar1=-1,
                                        scalar2=0x5f3759df,
                                        op0=ALU.mult, op1=ALU.add)
                tny = statp.tile([P, HSUB], f32, tag="tny")
                nc.vector.tensor_tensor(tny, y, y, ALU.mult)
                nc.vector.tensor_tensor(tny, tny, var, ALU.mult)
                nc.vector.tensor_scalar(out=tny, in0=tny, scalar1=-1.0,
                                        scalar2=1.5,
                                        op0=ALU.mult, op1=ALU.add)
                nc.vector.tensor_tensor(y, y, tny, ALU.mult)
                mr = statp.tile([P, 2 * HSUB], bf16, tag="mr")
                nc.vector.tensor_copy(mr[:, 0:HSUB], mvh[:, :, 0])
                nc.vector.tensor_scalar(out=mr[:, HSUB:2 * HSUB], in0=y,
                                        scalar1=SX, scalar2=None,
                                        op0=ALU.mult)
                mrt = pstp.tile([2 * HSUB, P], bf16, tag="mrt")
                nc.tensor.transpose(mrt, mr, eye_sb)
                mrs = statp.tile([2 * HSUB, P], bf16, tag="mrs")
                nc.scalar.activation(mrs, mrt, AF.Copy)
                hs = slice(h * HB, (h + 1) * HB)
                nc.gpsimd.dma_start(statd[0:2, hs], mrs)
                mu_bh = bcp.tile([P, HB], bf16, tag="mu_b", name="mu_b")
                nc.sync.dma_start(
                    mu_bh, statd[0:1, hs].to_broadcast((P, HB)))
                rs_bh = bcp.tile([P, HB], bf16, tag="rs_b", name="rs_b")
                nc.sync.dma_start(
                    rs_bh, statd[1:2, hs].to_broadcast((P, HB)))
                return mu_bh, rs_bh

            def phase2_half(b, xtps, bc, h):
                """T-space normalize half h to fp8 pair tiles."""
                mu_bh, rs_bh = bc
                xn8s = []
                for i in range(DP):
                    xn8 = xn8p.tile([P, 2, HB], fp8, tag="xn8", name="xn8")
                    for j in range(2):
                        nc.vector.tensor_tensor(xtps[h][i][:, j],
                                                xtps[h][i][:, j], mu_bh,
                                                ALU.subtract)
                        nc.vector.tensor_tensor(xn8[:, j],
                                                xtps[h][i][:, j], rs_bh,
                                                ALU.mult)
                    xn8s.append(xn8)
                return xn8s

            def phase3_half(b, xn8s_h, h, zc):
                """fp8 DoubleRow matmul1 + tanh + scores + exp for half h."""
                epks = []
                for cc in range(2):
                    c = h * 2 + cc
                    cs = slice(cc * CHUNK, (cc + 1) * CHUNK)
                    h8s = [h8p.tile([P, 2, CHUNK], fp8, tag="h8", name="h8")
                           for _ in range(EP)]
                    for e in range(ET):
                        ps = psmm.tile([P, CHUNK], f32, tag="psmm")
                        for i in range(DP):
                            nc.tensor.matmul(
                                ps, w1_sb[:, i, e, :],
                                xn8s_h[i][:, :, cs],
                                start=(i == 0), stop=(i == DP - 1),
                                perf_mode=DRSWI)
                        nc.scalar.activation(h8s[e // 2][:, e % 2, :], ps,
                                             AF.Tanh, bias=c2_sb[:, e:e + 1],
                                             scale=MM1_SCALE)
                    ps_sc = pssc.tile([1, CHUNK], f32, tag="pssc")
                    for k in range(EP):
                        nc.tensor.matmul(ps_sc,
                                         w2_sb[:, 2 * k:2 * k + 2, 0:1],
                                         h8s[k], start=(k == 0),
                                         stop=(k == EP - 1), perf_mode=DR)
                    ec = ecp.tile([1, CHUNK], bf16, tag="ec", name="ec")
                    nc.scalar.activation(ec, ps_sc, AF.Exp,
                                         bias=b2_sb[0:1, 0:1], scale=SC_SCALE,
                                         accum_out=zc[:, c:c + 1])
                    eb = dramp.tile([1, CHUNK], bf16, tag="eb", name="eb")
                    nc.scalar.dma_start(eb, ec)
                    epk = epkp.tile([P, NCHUNK], bf16, tag="epk", name="epk")
                    nc.scalar.dma_start(
                        epk, eb.rearrange("o (t p) -> (o p) t", p=P))
                    epks.append(epk)
                return epks

            def pool_half(xb, epks2, pp0, pp1, h):
                """Pooling matmuls for one half's chunks (one accumulation
                group spanning both halves: start at tt==0, stop at 15)."""
                for cc in range(2):
                    c = h * 2 + cc
                    for t in range(NG):
                        tt = c * NG + t
                        nc.tensor.matmul(pp0, epks2[cc][:, t:t + 1],
                                         xb[:, tt, 0:512],
                                         start=(tt == 0), stop=(tt == SUBT - 1))
                        nc.tensor.matmul(pp1, epks2[cc][:, t:t + 1],
                                         xb[:, tt, 512:1024],
                                         start=(tt == 0), stop=(tt == SUBT - 1))

            def phase4(b, xb, zc, epks, pps=None):
                """Pooling matmuls vs SBUF-kept x[s,d], divide by Z, store."""
                if pps is None:
                    pp0 = pspool.tile([1, CHUNK], f32, tag="pspool", name="pp0")
                    pp1 = pspool.tile([1, CHUNK], f32, tag="pspool", name="pp1")
                    pool_half(xb, epks[0:2], pp0, pp1, 0)
                    pool_half(xb, epks[2:4], pp0, pp1, 1)
                else:
                    pp0, pp1 = pps
                zt = zp.tile([1, 1], f32, tag="zt")
                nc.vector.tensor_reduce(zt, zc,
                                        axis=mybir.AxisListType.X, op=ALU.add)
                rz = zp.tile([1, 1], f32, tag="rz")
                nc.vector.reciprocal(rz, zt)
                ob = obp.tile([1, D], f32, tag="ob")
                nc.scalar.activation(ob[:, 0:512], pp0, AF.Copy,
                                     scale=rz[0:1, 0:1])
                nc.scalar.activation(ob[:, 512:1024], pp1, AF.Copy,
                                     scale=rz[0:1, 0:1])
                nc.sync.dma_start(out.ap()[b:b + 1, :], ob)

            prev = None
            for b in range(BLOC):
                last = False      # A/B: last-batch pooling split was neutral
                xb, xtps, statd, late_loads = phase1_loads(b)
                bc0 = phase1_stats(b, xb, statd, 0)
                late_loads()
                bc1 = phase1_stats(b, xb, statd, 1)
                if prev is not None:
                    phase4(*prev)
                zc = zp.tile([1, NCHUNK], f32, tag="zc", name="zc")
                xn8_h0 = phase2_half(b, xtps, bc0, 0)
                epks = phase3_half(b, xn8_h0, 0, zc)
                xn8_h1 = phase2_half(b, xtps, bc1, 1)
                if last:
                    # drain the last batch faster: pool h0 while h1 computes
                    pp0 = pspool.tile([1, CHUNK], f32, tag="pspool",
                                      name="pp0")
                    pp1 = pspool.tile([1, CHUNK], f32, tag="pspool",
                                      name="pp1")
                    pool_half(xb, epks, pp0, pp1, 0)
                epks += phase3_half(b, xn8_h1, 1, zc)
                if last:
                    pool_half(xb, epks[2:4], pp0, pp1, 1)
                    prev = (b, xb, zc, epks, (pp0, pp1))
                else:
                    prev = (b, xb, zc, epks)
            phase4(*prev)

    nc.compile()
    return nc


_NC_CACHE = {}


def _get_nc():
    if "nc" not in _NC_CACHE:
        _NC_CACHE["nc"] = build_nc()
    return _NC_CACHE["nc"]


def _prep_host(ln_gamma, ln_beta, W1, b1, W2, b2):
    import ml_dtypes
    f8 = ml_dtypes.float8_e4m3fn
    W1p = (np.asarray(ln_gamma, np.float32)[:, None]
           * np.asarray(W1, np.float32))
    w1q = np.clip(W1p * SW, -448, 448).astype(f8)
    # interleave for DoubleRowSwInterleave: per (pair i, e-tile) the 256
    # weight bytes per partition are [A_127 B_127 ... A_0 B_0] where
    # A/B are the even/odd d-tiles of the pair and columns are reversed
    b4 = w1q.reshape(DT, P, ET, P)            # [t, p, e, m]
    A, Bm = b4[0::2], b4[1::2]                # [DP, p, e, m]
    wv = np.empty((DP, P, ET, 2 * P), f8)
    wv[..., 0::2] = A[..., ::-1]
    wv[..., 1::2] = Bm[..., ::-1]
    w1s = np.ascontiguousarray(
        wv.transpose(1, 0, 2, 3).reshape(P, DP * ET * 2 * P))
    c2 = (np.asarray(ln_beta, np.float32) @ np.asarray(W1, np.float32)
          + np.asarray(b1, np.float32))
    c2pm = np.ascontiguousarray(c2.reshape(ET, P).T)          # [P, ET]
    w2q = np.clip(np.asarray(W2, np.float32)[:, 0] * SW,
                  -448, 448).astype(f8)
    w2pm = np.ascontiguousarray(w2q.reshape(ET, P).T)         # [P, ET]
    b2s = np.asarray(b2, np.float32).reshape(1, 1)
    return w1s, c2pm, w2pm, b2s


def run_cores(inputs, trace=False, **kw):
    import ml_dtypes
    x = np.asarray(inputs["x"], np.float32)
    w1q, c2, w2q, b2s = _prep_host(inputs["ln_gamma"], inputs["ln_beta"],
                                   inputs["W1"], inputs["b1"],
                                   inputs["W2"], inputs["b2"])
    xb16 = x.astype(ml_dtypes.bfloat16)          # [B, S, D]
    xt16 = xb16.transpose(0, 2, 1)               # [B, D, S] view
    nc = _get_nc()
    in_maps = []
    for c in range(NCORES):
        cb = slice(c * BLOC, (c + 1) * BLOC)
        # partition-major relayouts: [P, b, t, d] and [P, b, i, u, s]
        shard = np.ascontiguousarray(
            xb16[cb].reshape(BLOC, SUBT, P, D).transpose(2, 0, 1, 3)
        ).reshape(P, BLOC * SUBT * D)
        shardT = np.ascontiguousarray(
            xt16[cb].reshape(BLOC, DP, 2, P, S).transpose(3, 0, 1, 2, 4)
        ).reshape(P, BLOC * DP * 2 * S)
        in_maps.append(dict(xbf=shard, xt=shardT, w1q=w1q, w2q=w2q,
                            c2v=c2, b2s=b2s,
                            eye=np.eye(P, dtype=ml_dtypes.bfloat16)))
    res = run_bass_kernel_spmd(nc, in_maps, core_ids=list(range(NCORES)),
                               trace=trace, **kw)
    full = np.concatenate([res.results[c]["out"] for c in range(NCORES)], axis=0)
    return full, res


def kernel(**inputs) -> np.ndarray:
    out, _ = run_cores(inputs, trace=False)
    return out.astype(np.float32)
